# revision 31
# baseline (speedup 1.0000x reference)
"""LoRA MultiheadAttention on 8 Trainium2 NeuronCores (Bass/Tile), v5.

Sharding: core c = (batch n = c//2, head-group hg = c%2); each core handles
6 of 12 heads for one of 4 batches. LoRA folded into projection weights on
host (exact). Inputs shipped pre-transposed (E-major) in fp16.

Per-core pipeline (engine-balanced, software-pipelined):
  - q/k projections in f16 (PE, DVE bias); e0 upfront, e1/e2 + the whole v
    projection interleaved into pair-0's attention window. log2(e) folded
    into Wq/bq so PSUM scores arrive as z = log2e * (q.k).
  - scores: two heads packed concurrently in the PE array (row tiling,
    K=64 at partitions 0-63 / 64-127), [h1|h2] 512-col halves of one
    [128,1024] PSUM unit.
  - exp: split ACT (native exp -> fp8e4m3) / DVE (Schraudolph: z + 56.04
    -> int8, bit-punned e4m3) with finely interleaved unit assignment so
    neither engine starves on the 3-deep PSUM ring.
  - attnV: fp8 matmuls, M=128 = 64 replicated ones (denominator) + 64 v
    dims; 16-matmul chains per (head, l-quarter), one chain per 2 score
    s-tiles, double-buffered in 2 single-bank PSUM accumulators.
  - normalize: DVE reciprocal_approx_fast (base-0 denominator rows) +
    tensor_mul -> f16 oT.
  - out-projection f16; half-K partial output DMA'd as f16; host sums the
    two partials per batch and adds bias (pure unshard glue).
"""
import numpy as np

import concourse.bass as bass
import concourse.tile as tile
from concourse import bacc, mybir
from concourse.bass_utils import run_bass_kernel_spmd

L, N, E, H, R = 2048, 4, 768, 12, 16
ALPHA = 16.0
LORA_SCALE = ALPHA / R
HD = E // H          # 64
HG = 2               # head groups (column-parallel dimension)
HPG = H // HG        # 6 heads per core
EG = E // HG         # 384 columns per group
NC_ = 8
F32 = mybir.dt.float32
F16 = mybir.dt.float16
F8 = mybir.dt.float8e4
I8 = mybir.dt.int8
C1 = float(np.log2(np.e))                 # folded into Wq, bq on host
ACT_SCALE = float(np.log(2.0) / 8.0)      # exp(z*ACT_SCALE) = e^{score/8}
# Schraudolph bias: 8*(e4m3 bias 7) - 8*0.0573 (centering) + 0.5 (truncate
# -> round). DVE writes int8(z + DVE_BIAS), punned as e4m3 ~= exp.
DVE_BIAS = 56.0 - 8.0 * 0.0573 + 0.5

KC = E // 128        # 6 contraction chunks
LT = L // 128        # 16 s tiles
NPAIR = HPG // 2     # 3 head pairs per core

_CACHED = {}


def _build():
    nc = bacc.Bacc()
    xqT = nc.dram_tensor("xqT", [E, L], F16, kind="ExternalInput")
    xkT = nc.dram_tensor("xkT", [E, L], F16, kind="ExternalInput")
    xvT = nc.dram_tensor("xvT", [E, L], F16, kind="ExternalInput")
    wqT = nc.dram_tensor("wqT", [E, EG], F16, kind="ExternalInput")
    wkT = nc.dram_tensor("wkT", [E, EG], F16, kind="ExternalInput")
    wvT = nc.dram_tensor("wvT", [E, EG], F16, kind="ExternalInput")
    woT = nc.dram_tensor("woT", [EG, E], F16, kind="ExternalInput")
    bq = nc.dram_tensor("bq", [EG], F32, kind="ExternalInput")
    bk = nc.dram_tensor("bk", [EG], F32, kind="ExternalInput")
    out = nc.dram_tensor("out", [E, L], F16, kind="ExternalOutput")

    with tile.TileContext(nc) as tc:
        with (
            tc.tile_pool(name="persist", bufs=1) as persist,
            tc.tile_pool(name="ring4k", bufs=32) as ring4k,
            tc.tile_pool(name="small", bufs=2) as small,
            tc.tile_pool(name="psum", bufs=1, space="PSUM") as psum,
        ):
            # ---- small persistent state (x chunks DMA'd first, below) ----
            w16 = {}

            def dma_w(pname, wdram):
                for j in range(KC):
                    wt = persist.tile([128, EG], F16, name=f"w16_{pname}{j}")
                    nc.sync.dma_start(wt[:], wdram[j * 128:(j + 1) * 128, :])
                    w16[pname, j] = wt

            bias_t = {}
            for bname, bdram in (("q", bq), ("k", bk)):
                for j in range(NPAIR):
                    bt = persist.tile([128, 1], F32, name=f"b_{bname}{j}")
                    nc.sync.dma_start(bt[:], bdram[j * 128:(j + 1) * 128])
                    bias_t[bname, j] = bt

            # preload the exp activation table off the critical path
            warm = persist.tile([1, 1], F16, name="warm")
            nc.scalar.activation(warm[:], bias_t["q", 0][0:1, :],
                                 mybir.ActivationFunctionType.Exp,
                                 scale=ACT_SCALE)

            # v layout: [p, st(16), h(6), c(128)] e4m3, c = [64 ones | 64 v]
            # (ones columns land the replicated softmax denominator at psum
            # partitions 0-63, where the custom reciprocal wants base 0).
            vmega = persist.tile([128, LT * HPG * 128], F8, name="vmega")
            vm = vmega.rearrange("p (st h c) -> p st h c", st=LT, h=HPG)
            for h in range(HPG):
                ones_u32 = vm[:, :, h, 0:64].bitcast(mybir.dt.uint32)
                nc.vector.memset(ones_u32, 0x38383838)

            oT = [persist.tile([128, L], F16, name=f"oT{j}")
                  for j in range(NPAIR)]
            qkT = {}
            for pname in ("q", "k"):
                for e in range(NPAIR):
                    qkT[pname, e] = persist.tile(
                        [128, L], F16, name=f"{pname}T{e}")

            # ---- shared 4KB ring: x chunks, qkT, attn tiles ----
            x16 = {}

            def dma_x(pname, xdram):
                for j in range(KC):
                    xt = ring4k.tile([128, L], F16, tag="r4k", name="x16")
                    nc.sync.dma_start(xt[:], xdram[j * 128:(j + 1) * 128, :])
                    x16[pname, j] = xt

            def proj_qk(pname, e, lh, bias_eng):
                ps = psum.tile([128, 1024], F32, tag="sunit", bufs=3,
                               name="ps_proj")
                for half in range(2):
                    o_sl = ps[:, half * 512:(half + 1) * 512]
                    l0 = lh * 1024 + half * 512
                    for kk in range(KC):
                        nc.tensor.matmul(
                            o_sl,
                            w16[pname, kk][:, e * 128:(e + 1) * 128],
                            x16[pname, kk][:, l0:l0 + 512],
                            start=(kk == 0), stop=(kk == KC - 1),
                        )
                dst = qkT[pname, e][:, lh * 1024:(lh + 1) * 1024]
                if bias_eng == "act":
                    nc.scalar.activation(dst, ps[:],
                                         mybir.ActivationFunctionType.Identity,
                                         bias=bias_t[pname, e][:])
                else:
                    nc.vector.tensor_scalar_add(dst, ps[:],
                                                bias_t[pname, e][:])

            def proj_v(st):
                ps = psum.tile([128, 1024], F32, tag="sunit", bufs=3,
                               name="ps_vproj")
                for kk in range(KC):
                    nc.tensor.matmul(
                        ps[:, 0:EG],
                        x16["v", kk][:, st * 128:(st + 1) * 128],
                        w16["v", kk][:],
                        start=(kk == 0), stop=(kk == KC - 1),
                    )
                nc.vector.tensor_copy(
                    vm[:, st, :, 64:128],
                    ps[:, 0:EG].rearrange("p (h c) -> p h c", c=HD),
                )

            # ---- attention building blocks ----
            at_tiles = {}   # (pair, st) -> tile [128, 2, 2048] e4m3

            def scores_exp(p, st, lc, dve):
                ps = psum.tile([128, 1024], F32, tag="sunit", bufs=3,
                               name="ps_sc")
                for pos in range(2):
                    rows = slice(pos * 64, (pos + 1) * 64)
                    nc.tensor.matmul(
                        ps[:, pos * 512:(pos + 1) * 512],
                        qkT["k", p][rows, st * 128:(st + 1) * 128],
                        qkT["q", p][rows, lc * 512:(lc + 1) * 512],
                        start=True, stop=True,
                    )
                if (p, st) not in at_tiles:
                    at_tiles[p, st] = ring4k.tile(
                        [128, 2 * L], F8, tag="r4k", name="at2")
                r3 = at_tiles[p, st].rearrange("p (h l) -> p h l", h=2)
                out_ap = r3[:, :, lc * 512:(lc + 1) * 512]
                if dve:
                    nc.vector.tensor_scalar_add(
                        out_ap.bitcast(I8), ps[:], DVE_BIAS)
                else:
                    nc.scalar.activation(
                        out_ap, ps[:], mybir.ActivationFunctionType.Exp,
                        scale=ACT_SCALE)

            vchain = {}     # chain idx -> psum tile

            def attnv_mms(q_pair, pos, lhf, slot):
                """Quarter (slot 0..3) of the 64 accumulating fp8 matmuls for
                one (head, l-half) chain. The two l-blocks (separate PSUM
                banks) are paired on alternating row groups so consecutive
                matmuls stream concurrently."""
                key = (q_pair, pos, lhf)
                if slot == 0:
                    vchain[key] = psum.tile([128, 1024], F32, tag="vacc",
                                            bufs=1, name="ps_vt")
                vt = vchain[key]
                h = q_pair * 2 + pos
                for stc in range(4 * slot, 4 * slot + 4):
                    r3 = at_tiles[q_pair, stc].rearrange(
                        "p (h l) -> p h l", h=2)
                    for lb in range(2):
                        l0 = lhf * 1024 + lb * 512
                        nc.tensor.matmul(
                            vt[:, lb * 512:(lb + 1) * 512],
                            vm[:, stc, h, :],
                            r3[:, pos, l0:l0 + 512],
                            start=(stc == 0), stop=(stc == LT - 1),
                        )

            def attnv_norm(q_pair, pos, lhf):
                vt = vchain.pop((q_pair, pos, lhf))
                rt = small.tile([64, 1024], F32, tag="rt", bufs=2, name="rt")
                nc.vector.reciprocal_approx_fast(rt[:], vt[0:64, :])
                nc.vector.tensor_mul(
                    oT[q_pair][pos * 64:(pos + 1) * 64,
                               lhf * 1024:(lhf + 1) * 1024],
                    vt[64:128, :], rt[:])

            def outproj(eo, lh, copy_eng):
                po = psum.tile([128, 1024], F32, tag="sunit", bufs=3,
                               name="ps_out")
                for half in range(2):
                    l0 = lh * 1024 + half * 512
                    for j in range(NPAIR):
                        nc.tensor.matmul(
                            po[:, half * 512:(half + 1) * 512],
                            wo16[j][:, eo * 128:(eo + 1) * 128],
                            oT[j][:, l0:l0 + 512],
                            start=(j == 0), stop=(j == NPAIR - 1),
                        )
                osb = small.tile([128, 1024], F16, tag="osb", bufs=2,
                                 name="osb")
                if copy_eng == "act":
                    nc.scalar.copy(osb[:], po[:])
                else:
                    nc.vector.tensor_copy(osb[:], po[:])
                nc.sync.dma_start(
                    out[eo * 128:(eo + 1) * 128,
                        lh * 1024:(lh + 1) * 1024], osb[:])

            # ---- head phase: x DMA + e0 projections ----
            dma_w("q", wqT)
            dma_x("q", xqT)
            proj_qk("q", 0, 0, "dve")
            proj_qk("q", 0, 1, "dve")
            dma_w("k", wkT)
            dma_x("k", xkT)
            proj_qk("k", 0, 0, "dve")
            proj_qk("k", 0, 1, "dve")
            dma_w("v", wvT)
            dma_x("v", xvT)
            wo16 = []
            for j in range(NPAIR):
                wt = persist.tile([128, E], F16, name=f"wo16_{j}")
                nc.sync.dma_start(wt[:], woT[j * 128:(j + 1) * 128, :])
                wo16.append(wt)

            # remaining projection units interleaved into pair-0's window
            # (x chunks must die before the attn-tile ring recycles them)
            p0_extra = [("q", 1, 0), ("q", 1, 1), ("k", 1, 0), ("k", 1, 1),
                        ("q", 2, 0), ("q", 2, 1), ("k", 2, 0), ("k", 2, 1)]

            # attnV chain order: l-half-major for the tail pair so the
            # out-projection of each l-half can start as soon as possible
            CHAINS = [(0, 0), (1, 0), (0, 1), (1, 1)]  # (pos, l-half)

            # ---- main attention loop ----
            for p in range(NPAIR):
                for st in range(LT):
                    for lc in range(4):
                        slot = st * 4 + lc
                        if p == 0:
                            dve = (slot % 3 == 1)
                        else:
                            dve = (slot % 5) in (1, 3)
                        scores_exp(p, st, lc, dve)
                    if p == 0:
                        proj_v(st)
                        if st % 2 == 1 and st < 8:
                            for u in range(2):
                                pname, e, lh = p0_extra[(st - 1) + u]
                                proj_qk(pname, e, lh,
                                        "act" if u == 0 else "dve")
                    else:
                        c = st // 4
                        if st % 4 == 0 and c >= 1:
                            # free chain c-1's accumulator before chain c
                            # claims the single vacc PSUM slot
                            attnv_norm(p - 1, *CHAINS[c - 1])
                        attnv_mms(p - 1, CHAINS[c][0], CHAINS[c][1],
                                  slot=st % 4)
                if p > 0:
                    attnv_norm(p - 1, *CHAINS[3])

            # ---- tail: last pair's attnV + out-projection ----
            for ci, (pos, lhf) in enumerate(CHAINS):
                if ci >= 1:
                    attnv_norm(NPAIR - 1, *CHAINS[ci - 1])
                for slot in range(4):
                    attnv_mms(NPAIR - 1, pos, lhf, slot)
                if ci == 2:
                    # l-half 0 fully normalized once chain 1's norm emitted
                    for eo in range(6):
                        outproj(eo, 0, "act" if eo % 2 == 0 else "dve")
            attnv_norm(NPAIR - 1, *CHAINS[3])
            for eo in range(6):
                outproj(eo, 1, "act" if eo % 2 == 0 else "dve")
    nc.finalize()
    return nc


def kernel(query, key, value, in_proj_weight, in_proj_bias,
           q_down, q_up, k_down, k_up, v_down, v_up,
           out_proj_weight, out_proj_bias, out_down, out_up):
    if "nc" not in _CACHED:
        _CACHED["nc"] = _build()
    nc = _CACHED["nc"]

    f = np.float32
    h = np.float16
    # fold LoRA into the projection weights (exact algebraic identity)
    w_eff = {}
    for i, (dn, up) in enumerate(((q_down, q_up), (k_down, k_up),
                                  (v_down, v_up))):
        w = in_proj_weight[i * E:(i + 1) * E].astype(f)
        w_eff[i] = w + LORA_SCALE * (up.astype(f) @ dn.astype(f))
    wo_eff = out_proj_weight.astype(f) + LORA_SCALE * (
        out_up.astype(f) @ out_down.astype(f))
    # fold log2(e) into Wq / bq so device scores are log2e * (q.k)
    wq_s = C1 * w_eff[0]
    bq_s = C1 * in_proj_bias[0:E].astype(f)

    in_maps = []
    for c in range(NC_):
        n, hg = c // 2, c % 2
        sl = slice(hg * EG, (hg + 1) * EG)
        m = {
            "xqT": np.ascontiguousarray(query[:, n, :].T, dtype=h),
            "xkT": np.ascontiguousarray(key[:, n, :].T, dtype=h),
            "xvT": np.ascontiguousarray(value[:, n, :].T, dtype=h),
            "wqT": np.ascontiguousarray(wq_s[sl].T, dtype=h),
            "wkT": np.ascontiguousarray(w_eff[1][sl].T, dtype=h),
            "wvT": np.ascontiguousarray(w_eff[2][sl].T, dtype=h),
            "woT": np.ascontiguousarray(wo_eff[:, sl].T, dtype=h),
            "bq": np.ascontiguousarray(bq_s[sl], dtype=f),
            "bk": np.ascontiguousarray(in_proj_bias[E:2 * E][sl], dtype=f),
        }
        in_maps.append(m)

    _CACHED["in_maps"] = in_maps
    res = run_bass_kernel_spmd(nc, in_maps, list(range(NC_)))
    outp = np.empty((L, N, E), dtype=np.float32)
    bo_total = out_proj_bias.astype(f) + wo_eff @ np.ascontiguousarray(
        in_proj_bias[2 * E:3 * E], dtype=f)
    for n in range(N):
        outp[:, n, :] = (res.results[2 * n]["out"].astype(f)
                         + res.results[2 * n + 1]["out"].astype(f)).T + bo_total
    return outp


# revision 34
# speedup vs baseline: 1.0869x; 1.0869x over previous
"""LoRA MultiheadAttention on 8 Trainium2 NeuronCores (Bass/Tile), v5.

Sharding: core c = (batch n = c//2, head-group hg = c%2); each core handles
6 of 12 heads for one of 4 batches. LoRA folded into projection weights on
host (exact). Inputs shipped pre-transposed (E-major) in fp16.

Per-core pipeline (engine-balanced, software-pipelined):
  - q/k projections in f16 (PE, DVE bias); e0 upfront, e1/e2 + the whole v
    projection interleaved into pair-0's attention window. log2(e) folded
    into Wq/bq so PSUM scores arrive as z = log2e * (q.k).
  - scores: two heads packed concurrently in the PE array (row tiling,
    K=64 at partitions 0-63 / 64-127), [h1|h2] 512-col halves of one
    [128,1024] PSUM unit.
  - exp: split ACT (native exp -> fp8e4m3) / DVE (Schraudolph: z + 56.04
    -> int8, bit-punned e4m3) with finely interleaved unit assignment so
    neither engine starves on the 3-deep PSUM ring.
  - attnV: fp8 matmuls, M=128 = 64 replicated ones (denominator) + 64 v
    dims; 16-matmul chains per (head, l-quarter), one chain per 2 score
    s-tiles, double-buffered in 2 single-bank PSUM accumulators.
  - normalize: DVE reciprocal_approx_fast (base-0 denominator rows) +
    tensor_mul -> f16 oT.
  - out-projection f16; half-K partial output DMA'd as f16; host sums the
    two partials per batch and adds bias (pure unshard glue).
"""
import numpy as np

import concourse.bass as bass
import concourse.tile as tile
from concourse import bacc, mybir
from concourse.bass_utils import run_bass_kernel_spmd

L, N, E, H, R = 2048, 4, 768, 12, 16
ALPHA = 16.0
LORA_SCALE = ALPHA / R
HD = E // H          # 64
HG = 2               # head groups (column-parallel dimension)
HPG = H // HG        # 6 heads per core
EG = E // HG         # 384 columns per group
NC_ = 8
F32 = mybir.dt.float32
F16 = mybir.dt.float16
F8 = mybir.dt.float8e4
I8 = mybir.dt.int8
C1 = float(np.log2(np.e))                 # folded into Wq, bq on host
ACT_SCALE = float(np.log(2.0) / 8.0)      # exp(z*ACT_SCALE) = e^{score/8}
# Schraudolph bias: 8*(e4m3 bias 7) - 8*0.0573 (centering) + 0.5 (truncate
# -> round). DVE writes int8(z + DVE_BIAS), punned as e4m3 ~= exp.
DVE_BIAS = 56.0 - 8.0 * 0.0573 + 0.5

KC = E // 128        # 6 contraction chunks
LT = L // 128        # 16 s tiles
NPAIR = HPG // 2     # 3 head pairs per core

_CACHED = {}


def _build():
    nc = bacc.Bacc()
    xqT = nc.dram_tensor("xqT", [E, L], F16, kind="ExternalInput")
    xkT = nc.dram_tensor("xkT", [E, L], F16, kind="ExternalInput")
    xvT = nc.dram_tensor("xvT", [E, L], F16, kind="ExternalInput")
    wqT = nc.dram_tensor("wqT", [E, EG], F16, kind="ExternalInput")
    wkT = nc.dram_tensor("wkT", [E, EG], F16, kind="ExternalInput")
    wvT = nc.dram_tensor("wvT", [E, EG], F16, kind="ExternalInput")
    woT = nc.dram_tensor("woT", [EG, E], F16, kind="ExternalInput")
    bq = nc.dram_tensor("bq", [EG], F32, kind="ExternalInput")
    bk = nc.dram_tensor("bk", [EG], F32, kind="ExternalInput")
    out = nc.dram_tensor("out", [E, L], F16, kind="ExternalOutput")

    with tile.TileContext(nc) as tc:
        with (
            tc.tile_pool(name="persist", bufs=1) as persist,
            tc.tile_pool(name="ring4k", bufs=32) as ring4k,
            tc.tile_pool(name="small", bufs=2) as small,
            tc.tile_pool(name="psum", bufs=1, space="PSUM") as psum,
        ):
            # ---- small persistent state (x chunks DMA'd first, below) ----
            w16 = {}

            def dma_w(pname, wdram):
                for j in range(KC):
                    wt = persist.tile([128, EG], F16, name=f"w16_{pname}{j}")
                    nc.sync.dma_start(wt[:], wdram[j * 128:(j + 1) * 128, :])
                    w16[pname, j] = wt

            bias_t = {}
            for bname, bdram in (("q", bq), ("k", bk)):
                for j in range(NPAIR):
                    bt = persist.tile([128, 1], F32, name=f"b_{bname}{j}")
                    nc.sync.dma_start(bt[:], bdram[j * 128:(j + 1) * 128])
                    bias_t[bname, j] = bt

            # preload the exp activation table off the critical path
            warm = persist.tile([1, 1], F16, name="warm")
            nc.scalar.activation(warm[:], bias_t["q", 0][0:1, :],
                                 mybir.ActivationFunctionType.Exp,
                                 scale=ACT_SCALE)

            # v layout: [p, st(16), h(6), c(128)] e4m3, c = [64 ones | 64 v]
            # (ones columns land the replicated softmax denominator at psum
            # partitions 0-63, where the custom reciprocal wants base 0).
            vmega = persist.tile([128, LT * HPG * 128], F8, name="vmega")
            vm = vmega.rearrange("p (st h c) -> p st h c", st=LT, h=HPG)
            for h in range(HPG):
                ones_u32 = vm[:, :, h, 0:64].bitcast(mybir.dt.uint32)
                nc.vector.memset(ones_u32, 0x38383838)

            oT = [persist.tile([128, L], F16, name=f"oT{j}")
                  for j in range(NPAIR)]
            qkT = {}
            for pname in ("q", "k"):
                for e in range(NPAIR):
                    qkT[pname, e] = persist.tile(
                        [128, L], F16, name=f"{pname}T{e}")

            # ---- shared 4KB ring: x chunks, qkT, attn tiles ----
            x16 = {}

            def dma_x(pname, xdram):
                for j in range(KC):
                    xt = ring4k.tile([128, L], F16, tag="r4k", name="x16")
                    nc.sync.dma_start(xt[:], xdram[j * 128:(j + 1) * 128, :])
                    x16[pname, j] = xt

            def proj_qk(pname, e, lh, bias_eng):
                ps = psum.tile([128, 1024], F32, tag="sunit", bufs=3,
                               name="ps_proj")
                for half in range(2):
                    o_sl = ps[:, half * 512:(half + 1) * 512]
                    l0 = lh * 1024 + half * 512
                    for kk in range(KC):
                        nc.tensor.matmul(
                            o_sl,
                            w16[pname, kk][:, e * 128:(e + 1) * 128],
                            x16[pname, kk][:, l0:l0 + 512],
                            start=(kk == 0), stop=(kk == KC - 1),
                        )
                dst = qkT[pname, e][:, lh * 1024:(lh + 1) * 1024]
                if bias_eng == "act":
                    nc.scalar.activation(dst, ps[:],
                                         mybir.ActivationFunctionType.Identity,
                                         bias=bias_t[pname, e][:])
                else:
                    nc.vector.tensor_scalar_add(dst, ps[:],
                                                bias_t[pname, e][:])

            def proj_v(st):
                ps = psum.tile([128, 1024], F32, tag="sunit", bufs=3,
                               name="ps_vproj")
                for kk in range(KC):
                    nc.tensor.matmul(
                        ps[:, 0:EG],
                        x16["v", kk][:, st * 128:(st + 1) * 128],
                        w16["v", kk][:],
                        start=(kk == 0), stop=(kk == KC - 1),
                    )
                nc.vector.tensor_copy(
                    vm[:, st, :, 64:128],
                    ps[:, 0:EG].rearrange("p (h c) -> p h c", c=HD),
                )

            # ---- attention building blocks ----
            at_tiles = {}   # (pair, st) -> tile [128, 2, 2048] e4m3

            def scores_exp(p, st, lc, dve):
                ps = psum.tile([128, 1024], F32, tag="sunit", bufs=3,
                               name="ps_sc")
                for pos in range(2):
                    rows = slice(pos * 64, (pos + 1) * 64)
                    nc.tensor.matmul(
                        ps[:, pos * 512:(pos + 1) * 512],
                        qkT["k", p][rows, st * 128:(st + 1) * 128],
                        qkT["q", p][rows, lc * 512:(lc + 1) * 512],
                        start=True, stop=True,
                    )
                if (p, st) not in at_tiles:
                    at_tiles[p, st] = ring4k.tile(
                        [128, 2 * L], F8, tag="r4k", name="at2")
                r3 = at_tiles[p, st].rearrange("p (h l) -> p h l", h=2)
                out_ap = r3[:, :, lc * 512:(lc + 1) * 512]
                if dve:
                    nc.vector.tensor_scalar_add(
                        out_ap.bitcast(I8), ps[:], DVE_BIAS)
                else:
                    nc.scalar.activation(
                        out_ap, ps[:], mybir.ActivationFunctionType.Exp,
                        scale=ACT_SCALE)

            vchain = {}     # chain idx -> psum tile

            def attnv_mms(q_pair, pos, qt, first_half):
                """8 of the 16 accumulating fp8 matmuls for one
                (head, l-quarter) chain."""
                key = (q_pair, pos, qt)
                if first_half:
                    vchain[key] = psum.tile([128, 512], F32, tag="vacc",
                                            bufs=2, name="ps_vt")
                vt = vchain[key]
                h = q_pair * 2 + pos
                l0 = qt * 512
                sts = range(0, 8) if first_half else range(8, 16)
                for stc in sts:
                    r3 = at_tiles[q_pair, stc].rearrange(
                        "p (h l) -> p h l", h=2)
                    nc.tensor.matmul(
                        vt[:], vm[:, stc, h, :], r3[:, pos, l0:l0 + 512],
                        start=(stc == 0), stop=(stc == LT - 1),
                    )

            def attnv_norm(q_pair, pos, qt):
                vt = vchain.pop((q_pair, pos, qt))
                rt = small.tile([64, 512], F32, tag="rt", bufs=2, name="rt")
                nc.vector.reciprocal_approx_fast(rt[:], vt[0:64, :])
                nc.vector.tensor_mul(
                    oT[q_pair][pos * 64:(pos + 1) * 64,
                               qt * 512:(qt + 1) * 512],
                    vt[64:128, :], rt[:])

            def outproj(eo, lh, copy_eng):
                po = psum.tile([128, 1024], F32, tag="sunit", bufs=3,
                               name="ps_out")
                for half in range(2):
                    l0 = lh * 1024 + half * 512
                    for j in range(NPAIR):
                        nc.tensor.matmul(
                            po[:, half * 512:(half + 1) * 512],
                            wo16[j][:, eo * 128:(eo + 1) * 128],
                            oT[j][:, l0:l0 + 512],
                            start=(j == 0), stop=(j == NPAIR - 1),
                        )
                osb = small.tile([128, 1024], F16, tag="osb", bufs=2,
                                 name="osb")
                if copy_eng == "act":
                    nc.scalar.copy(osb[:], po[:])
                else:
                    nc.vector.tensor_copy(osb[:], po[:])
                nc.sync.dma_start(
                    out[eo * 128:(eo + 1) * 128,
                        lh * 1024:(lh + 1) * 1024], osb[:])

            # ---- head phase: x DMA + e0 projections ----
            dma_w("q", wqT)
            dma_x("q", xqT)
            proj_qk("q", 0, 0, "dve")
            proj_qk("q", 0, 1, "dve")
            dma_w("k", wkT)
            dma_x("k", xkT)
            proj_qk("k", 0, 0, "dve")
            proj_qk("k", 0, 1, "dve")
            dma_w("v", wvT)
            dma_x("v", xvT)
            wo16 = []
            for j in range(NPAIR):
                wt = persist.tile([128, E], F16, name=f"wo16_{j}")
                nc.sync.dma_start(wt[:], woT[j * 128:(j + 1) * 128, :])
                wo16.append(wt)

            # remaining projection units interleaved into pair-0's window
            # (x chunks must die before the attn-tile ring recycles them)
            p0_extra = [("q", 1, 0), ("q", 1, 1), ("k", 1, 0), ("k", 1, 1),
                        ("q", 2, 0), ("q", 2, 1), ("k", 2, 0), ("k", 2, 1)]

            # attnV chain order: l-half-major for the tail pair so the
            # out-projection of each l-half can start as soon as possible
            CHAINS = [(0, 0), (1, 0), (0, 1), (1, 1),
                      (0, 2), (1, 2), (0, 3), (1, 3)]  # (pos, l-quarter)

            # ---- main attention loop ----
            for p in range(NPAIR):
                for st in range(LT):
                    for lc in range(4):
                        slot = st * 4 + lc
                        if p == 0:
                            dve = (slot % 3 == 1)
                        else:
                            dve = (slot % 5) in (1, 3)
                        scores_exp(p, st, lc, dve)
                    if p == 0:
                        proj_v(st)
                        if st < 8:
                            pname, e, lh = p0_extra[st]
                            proj_qk(pname, e, lh,
                                    "act" if st % 2 == 0 else "dve")
                    else:
                        c = st // 2
                        if st % 2 == 0 and c >= 2:
                            # free chain c-2's accumulator before chain c
                            # claims its PSUM slot (vacc ring of 2)
                            attnv_norm(p - 1, *CHAINS[c - 2])
                        attnv_mms(p - 1, CHAINS[c][0], CHAINS[c][1],
                                  first_half=(st % 2 == 0))
                if p > 0:
                    for c in range(6, 8):
                        attnv_norm(p - 1, *CHAINS[c])

            # ---- tail: last pair's attnV + out-projection ----
            for ci, (pos, qt) in enumerate(CHAINS):
                attnv_mms(NPAIR - 1, pos, qt, True)
                attnv_mms(NPAIR - 1, pos, qt, False)
                if ci >= 2:
                    attnv_norm(NPAIR - 1, *CHAINS[ci - 2])
                if ci == 5:
                    for eo in range(6):
                        outproj(eo, 0, "act" if eo % 2 == 0 else "dve")
            attnv_norm(NPAIR - 1, *CHAINS[6])
            attnv_norm(NPAIR - 1, *CHAINS[7])
            for eo in range(6):
                outproj(eo, 1, "act" if eo % 2 == 0 else "dve")
    nc.finalize()
    return nc


def kernel(query, key, value, in_proj_weight, in_proj_bias,
           q_down, q_up, k_down, k_up, v_down, v_up,
           out_proj_weight, out_proj_bias, out_down, out_up):
    if "nc" not in _CACHED:
        _CACHED["nc"] = _build()
    nc = _CACHED["nc"]

    f = np.float32
    h = np.float16
    # fold LoRA into the projection weights (exact algebraic identity)
    w_eff = {}
    for i, (dn, up) in enumerate(((q_down, q_up), (k_down, k_up),
                                  (v_down, v_up))):
        w = in_proj_weight[i * E:(i + 1) * E].astype(f)
        w_eff[i] = w + LORA_SCALE * (up.astype(f) @ dn.astype(f))
    wo_eff = out_proj_weight.astype(f) + LORA_SCALE * (
        out_up.astype(f) @ out_down.astype(f))
    # fold log2(e) into Wq / bq so device scores are log2e * (q.k)
    wq_s = C1 * w_eff[0]
    bq_s = C1 * in_proj_bias[0:E].astype(f)

    in_maps = []
    for c in range(NC_):
        n, hg = c // 2, c % 2
        sl = slice(hg * EG, (hg + 1) * EG)
        m = {
            "xqT": np.ascontiguousarray(query[:, n, :].T, dtype=h),
            "xkT": np.ascontiguousarray(key[:, n, :].T, dtype=h),
            "xvT": np.ascontiguousarray(value[:, n, :].T, dtype=h),
            "wqT": np.ascontiguousarray(wq_s[sl].T, dtype=h),
            "wkT": np.ascontiguousarray(w_eff[1][sl].T, dtype=h),
            "wvT": np.ascontiguousarray(w_eff[2][sl].T, dtype=h),
            "woT": np.ascontiguousarray(wo_eff[:, sl].T, dtype=h),
            "bq": np.ascontiguousarray(bq_s[sl], dtype=f),
            "bk": np.ascontiguousarray(in_proj_bias[E:2 * E][sl], dtype=f),
        }
        in_maps.append(m)

    _CACHED["in_maps"] = in_maps
    res = run_bass_kernel_spmd(nc, in_maps, list(range(NC_)))
    outp = np.empty((L, N, E), dtype=np.float32)
    bo_total = out_proj_bias.astype(f) + wo_eff @ np.ascontiguousarray(
        in_proj_bias[2 * E:3 * E], dtype=f)
    for n in range(N):
        outp[:, n, :] = (res.results[2 * n]["out"].astype(f)
                         + res.results[2 * n + 1]["out"].astype(f)).T + bo_total
    return outp
